# revision 86
# baseline (speedup 1.0000x reference)
"""Trainium2 Bass kernel: single-head attention module (dense transformer).

Computes, for x [4, 4096, 256] (f32) and per-projection weights/biases:
    q = x @ Wq + bq;  k = x @ Wk + bk;  v = x @ Wv + bv
    out = softmax((q k^T) / sqrt(256)) @ v @ Wo + bo

Sharding over 8 NeuronCores: core c handles batch c//2, query half c%2.
The host rotates each core's batch so its queries are always rows 0..2047
(softmax is key-order invariant), keeping the device program identical
across cores. Each core computes K/V for its whole batch (redundant with
its pair core, which is cheap) and attention + output projection for its
2048 queries.

Per-core kernel layout. Scores run in float32r (full-rate ~fp32 on the
PE, 1 col/cycle); the P@V and softmax-denominator matmuls run in
fp8-e4m3 DoubleRow mode (0.5 col/cycle, 2x PE rate) — the only
sub-f32r speedup the PE offers. Measured numerics: e4m3 P/V + bf16
x/Wqkv costs 1.49e-2 Frobenius rel err (quantizing Q/K to fp8 would
cost 2.9e-2 — over the 2e-2 budget — so scores stay f32r).
  - x^T arrives PRE-TRANSPOSED and PACKED from the host ([128, 2*S]
    bf16, both d-chunks per partition row): three wide DMAs, no PE
    transposes. wq/wk/wv are packed into one bf16 DMA (ACT's HWDGE
    ring, parallel to the x stream on SP's ring), bq/bk into another —
    the DMA queue chains issue-on-completion (~1.5us fixed per DMA in a
    ~3-deep flight window), so fewer/larger transfers shorten the
    input-critical path. ~190 dummy fp8 matmuls warm the PE's p-state
    ramp (0.65->2.4GHz over 3us continuous) under the DMA head.
  - Q^T [e, sq] / K^T [e, sk] produced directly transposed (lhsT = W
    chunk, moving = x^T). V in natural [sk, e], evicted PSUM->SBUF as
    e4m3 (the eviction converts for free). All persistent activations
    are split into [128,512]-column page tiles with exactly ONE writer
    each (the scheduler serializes cross-engine writes to a shared
    tile); evictions split per-half: DVE always h0, ACT always h1.
  - Scores for a k-tile PAIR land in one [128,1024] PSUM tile (2
    banks); ONE 1024-wide exp per pair (amortizes the ~370ns ACT fixed
    cost) writes P^T = exp(S^T/16 - 1.5) straight to an e4m3 SBUF
    tile. The -1.5 bias keeps max(P) ~ 96 < 240 (TRN e4m3 saturates to
    Inf above 240) and cancels exactly in the normalization. A dummy
    exp at t~1us pins the exp_and_others ACT table (identity/copy/exp
    share it) so no 1.3us table reload lands mid-stream.
  - P@V: per pair, two DoubleRow matmuls (e-halves) with stationary
    v8[k,2,e] and moving pt[k,2,q] accumulate out^T[e, 512q] over 16
    pairs; denominator: one DoubleRow matmul with an e4m3 ones
    stationary into accd. PV+denom are emitted TWO pairs behind scores
    so the ~1.4us exp release never stalls the PE. PSUM = 2x2 (scores)
    + 2 (acc) + 1 (accd) + 1 (final proj) = 8 banks exactly.
  - out^T is scaled by 1/denom (DVE) and fed as the stationary of the
    final f32r projection, landing output in natural [sq, f] layout
    for paired 256-row output DMAs. Final projections of block qb are
    interleaved into block qb+1's score stream; the last block flushes
    through the freed score banks with its two out-DMAs on different
    HWDGE rings. bo broadcasts via GPSIMD partition_broadcast (a PE
    ones-matmul would let the scheduler gate attention on the late bo
    DMA); bv folds into bo host-side (attention rows sum to 1).

Sim/HW exec: 123644 ns/core (baseline 169150; PE ~102us busy of which
~96us is real work: scores 54.6 + PV/denom 20.5 + projections 17.1 +
finals 3.4; ACT exp 66; the ~21us of PE idle is the DMA-bound head,
~120ns/pair exp-release slack in steady state, and the end drain).
"""

import numpy as np

import concourse.bass as bass  # noqa: F401  (AP types come through tile/bacc)
import concourse.tile as tile
from concourse import bacc, mybir
from concourse.bass_utils import run_bass_kernel_spmd

B, S, D = 4, 4096, 256
SQ = S // 2  # queries per core
NCORES = 8
F32 = mybir.dt.float32
F32R = mybir.dt.float32r
BF16 = mybir.dt.bfloat16
FP8 = mybir.dt.float8e4
U8 = mybir.dt.uint8
SCALE = 1.0 / 16.0  # 1/sqrt(D)
EXP_BIAS = -1.5  # exp(s/16 - 1.5): max scaled score ~6.1 -> max P ~ e^4.6=99
DR = mybir.MatmulPerfMode.DoubleRow


def _r(ap):
    """View an fp32 AP as float32r: full-rate fp32 matmul on the PE."""
    return ap.bitcast(F32R)


def _build(phases=3):
    nc = bacc.Bacc("TRN2", target_bir_lowering=False, debug=False,
                   num_devices=NCORES)

    # x and the packed Q/K/V weights arrive as bf16 (host converts): bf16
    # enables the XBAR DMA-transpose of x (2-byte dtypes only), halves the x
    # DMA traffic, and costs ~1e-3 rel err against the 2e-2 budget. Wo stays
    # f32 (its matmul partner o is f32r). wq/wk/wv are packed into ONE DRAM
    # tensor (and bq/bk likewise) because each dma_start costs ~650ns on the
    # sequencer + HWDGE AND the DMA queue chains issue on completion (~1.5us
    # fixed per DMA): fewer, larger DMAs shorten the input stream critically.
    # x arrives PRE-TRANSPOSED from the host ([D, S] bf16): x^T is what every
    # projection consumes, host transposition is free w.r.t. HW exec time,
    # and loading it with 4-6 plain wide DMAs beats 8 XBAR DMA-transposes on
    # the chained DMA queue (~1.5us fixed cost per DMA in flight-window 3).
    # Packed layout [128, 2*S]: partition p holds d-chunk0 row p then
    # d-chunk1 row p, so ONE wide DMA delivers both contraction chunks.
    xkvT_d = nc.dram_tensor("xkvT", [128, 2 * S], BF16,
                            kind="ExternalInput").ap()
    wqkv = nc.dram_tensor("wqkv", [3 * D, D], BF16, kind="ExternalInput").ap()
    wo_d = nc.dram_tensor("wo", [D, D], F32, kind="ExternalInput").ap()
    bqk = nc.dram_tensor("bqk", [2 * D], F32, kind="ExternalInput").ap()
    bo_d = nc.dram_tensor("bo", [D], F32, kind="ExternalInput").ap()
    out = nc.dram_tensor("out", [SQ, D], F32, kind="ExternalOutput").ap()

    bo_row = bo_d.rearrange("(a b) -> a b", a=1)  # [1, 256]
    bqk_pnc = bqk.rearrange("(n c p) -> p (n c)", n=2, p=128)  # [128, 4]
    wqkv_g = wqkv.rearrange("(n j p) c -> p n j c", n=3, j=2)  # [128,3,2,256]
    wo_g = wo_d.rearrange("(j p) c -> p j c", j=2)
    out_g = out.rearrange("(g j p) c -> g p j c", j=2, p=128)   # [8,128,2,256]

    with tile.TileContext(nc) as tc:
        with (
            tc.tile_pool(name="const", bufs=1) as cpool,
            tc.tile_pool(name="pt", bufs=4) as pt_pool,
            tc.tile_pool(name="ovec", bufs=2) as ovec_pool,
            tc.tile_pool(name="fout", bufs=2) as fout_pool,
            tc.tile_pool(name="psmm", bufs=1, space="PSUM") as psmm,
            tc.tile_pool(name="psacc", bufs=1, space="PSUM") as psacc,
        ):
            # ---- constants ----
            # e4m3 ones [128, 2*128] for the DoubleRow denominator matmul
            ones8 = cpool.tile([128, 256], FP8, tag="ones8", name="ones8")
            # memset on GPSIMD: lands ~0.5us earlier than DVE (it gates the
            # PE warm-up stream below).
            nc.gpsimd.memset(ones8[:].bitcast(U8), 0x38)  # e4m3 1.0
            ebias = cpool.tile([128, 1], F32, tag="ebias", name="ebias")
            nc.vector.memset(ebias[:], EXP_BIAS)
            # Dummy exp pins the exp_and_others ACT table now (~t=1us, during
            # the DMA head); identity/copy/exp all live in that set, so no
            # 1.3us table reload ever lands in front of the attention exps.
            scratch1 = cpool.tile([128, 1], F32, tag="scr1", name="scr1")
            nc.scalar.activation(scratch1[:], ebias[:],
                                 mybir.ActivationFunctionType.Exp)

            # ---- persistent activations, split into [128,512]-column pages
            # so every eviction writes exactly one page (single writer per
            # tile: the scheduler serializes cross-engine writes to a shared
            # tile, which would otherwise convoy the DVE/ACT eviction pairs).
            def pages(tag, n, dt=F32R):
                return [cpool.tile([128, 512], dt, tag=f"{tag}_{p}",
                                   name=f"{tag}_{p}") for p in range(n)]

            # x^T lives in three packed tiles (s-ranges 0:1024, 1024:2048,
            # 2048:4096; each holds both d-chunks side by side, matching the
            # packed DRAM layout): one DMA per tile, single writer. The
            # 0.5/0.5/1 MiB split gets K0/Q0 going ~1.5us earlier than two
            # 1 MiB slabs would while keeping the chained-DMA count low.
            XRANGES = [(0, 1024), (1024, 2048), (2048, 4096)]
            xkvTt = [cpool.tile([128, 2 * (b - a)], BF16, tag=f"xkvT_{i}",
                                name=f"xkvT_{i}")
                     for i, (a, b) in enumerate(XRANGES)]
            qTp = [pages(f"qT{c}", 4) for c in range(2)]
            kTp = [pages(f"kT{c}", 8) for c in range(2)]
            v8p = pages("v8", 16, dt=FP8)

            # wq/wk/wv in one packed bf16 tile [128, 3*2*256]; wo f32r.
            wqkv_sb = cpool.tile([128, 6 * D], BF16, tag="wqkv", name="wqkv")
            wo_sb = cpool.tile([128, 2 * D], F32R, tag="w_wo", name="w_wo")
            _widx = {"wq": 0, "wk": 1, "wv": 2}

            def wchunk(n, c):  # [128, 256] d-chunk c of W
                if n == "wo":
                    return wo_sb[:, c * D:(c + 1) * D]
                return wqkv_sb[:, (_widx[n] * 2 + c) * D:
                               (_widx[n] * 2 + c + 1) * D]

            # Packed biases: [128, 4] = (bq c0, bq c1, bk c0, bk c1).
            b4 = cpool.tile([128, 4], F32, tag="b4", name="b4")
            bqc = [b4[:, c:c + 1] for c in range(2)]
            bkc = [b4[:, 2 + c:3 + c] for c in range(2)]

            # ---- DMA issue order = transfer order (single serial HWDGE +
            # DMA-engine chain, ~3 DMAs in flight globally).
            xkvT_cs = xkvT_d.rearrange("p (c s) -> p c s", c=2)

            def dma_xT(i):
                a, b = XRANGES[i]
                nc.sync.dma_start(
                    xkvTt[i].rearrange("p (c s) -> p c s", c=2),
                    xkvT_cs[:, :, a:b])

            # Weights/biases go out on the ACT sequencer's HWDGE ring (TRN2
            # has two physical rings: qSPDynamicHW + qActDynamicHW), so their
            # issue chain runs in parallel with the x^T stream on SP.
            nc.scalar.dma_start(
                wqkv_sb.rearrange("p (n j c) -> p n j c", n=3, j=2),
                wqkv_g[:])
            nc.scalar.dma_start(b4[:], bqk_pnc)
            dma_xT(0)
            dma_xT(1)
            dma_xT(2)
            nc.sync.dma_start(
                wo_sb.rearrange("p (j c) -> p j c", j=2), _r(wo_g[:]))

            # bo broadcast across partitions on the (idle) GPSIMD engine:
            # row DMA [1,256] then partition-broadcast into both halves of
            # bob [128,512], so one wide add covers two output row-tiles.
            # No PE involvement — an fp32 ones-matmul here would let the
            # scheduler gate the attention stream on this late DMA. (bv is
            # folded into bo host-side: attention rows sum to 1.)
            bob = cpool.tile([128, 2 * D], F32, tag="bob", name="bob")
            row = cpool.tile([1, D], F32, tag="bor", name="bor")
            nc.sync.dma_start(row[:], bo_row[:])
            for half in range(2):
                nc.gpsimd.partition_broadcast(
                    bob[:, half * D:(half + 1) * D], row[:])

            # PE p-state warm-up: ~170 dummy DoubleRow matmuls on the ones8
            # tile keep the PE continuously busy from ~1.3us (after the ones8
            # memset) until the first x^T slab + weights land (~6us). The PE
            # clock ramps 0.65 -> 1.2 -> 2.4 GHz over 3us of CONTINUOUS
            # execution and resets on idle, so without this the whole first
            # ~3us of projections would run at half clock.
            ones8_3 = ones8[:].rearrange("p (two e) -> p two e", two=2)
            warm = psmm.tile([128, D], F32, tag="fp", name="fp", bufs=1)
            for _ in range(190):
                nc.tensor.matmul(warm[:, 0:128], ones8_3, ones8_3,
                                 start=True, stop=True, perf_mode=DR)

            # ---- phases 1+2, interleaved per x group ----
            # Each [128,1024] PSUM tile is processed as two independent
            # halves: matmuls for half h, then its eviction — half 0 always
            # on DVE, half 1 always on ACT, each into its own page tile. The
            # eviction of half 0 completes while the PE fills half 1, so the
            # 2-deep PSUM ring never stalls the PE.
            def evict1(dsl, src, h, bias=None):
                if bias is None:
                    if h == 0:
                        nc.vector.tensor_copy(dsl, src)
                    else:
                        nc.scalar.copy(dsl, src)
                elif h == 0:
                    nc.vector.tensor_scalar_add(dsl, src, bias)
                else:
                    nc.scalar.activation(
                        dsl, src, mybir.ActivationFunctionType.Identity,
                        bias=bias)

            def xT(c, col, width=512):  # read helper over x^T slabs
                for t, (a, b) in enumerate(XRANGES):
                    if col < b:
                        off = col - a
                        w = b - a
                        return xkvTt[t][:, c * w + off:c * w + off + width]

            def project_qk(wn, dstTp, bcol, g, et):
                # lhsT = W[d_chunk, e_tile], moving = x^T[d_chunk, group g].
                pp = psmm.tile([128, 1024], F32, tag="sp", name="sp",
                               bufs=2)
                for h in range(2):
                    blk = g * 2 + h
                    for c in range(2):
                        nc.tensor.matmul(
                            pp[:, h * 512:(h + 1) * 512],
                            wchunk(wn, c)[:, et * 128:(et + 1) * 128],
                            xT(c, blk * 512),
                            start=(c == 0), stop=(c == 1),
                        )
                    evict1(dstTp[et][blk][:],
                           pp[:, h * 512:(h + 1) * 512],
                           h, bias=bcol[et])

            def project_v(stp):
                # V natural layout [sk, e]; four sk-tiles per [128,1024]
                # PSUM tile; each half's eviction converts fp32 -> e4m3 into
                # one v8 page (= one k-tile pair, the PV stationary unit).
                vp = psmm.tile([128, 1024], F32, tag="sp", name="sp", bufs=2)
                for h in range(2):
                    for quad in (h * 2, h * 2 + 1):
                        st = stp * 4 + quad
                        for c in range(2):
                            nc.tensor.matmul(
                                vp[:, quad * D:(quad + 1) * D],
                                xT(c, st * 128, 128),
                                wchunk("wv", c),
                                start=(c == 0), stop=(c == 1),
                            )
                    evict1(v8p[stp * 2 + h][:],
                           vp[:, h * 512:(h + 1) * 512], h)

            # Strictly slab-major: all tiles reading x^T slab i are emitted
            # together, in DMA arrival order, so the PE consumes each slab
            # at the rate the chained DMA queue delivers them. (Interleaving
            # projections into the score stream was tried and loses: with the
            # 2-deep PSUM ring, consecutive score tiles end up 2 allocations
            # apart and an interposed projection tile provides less cover
            # than attention's own S+2xPV stream for the ~1.4us exp release.)
            if phases >= 2:
                project_qk("wk", kTp, bkc, 0, 0)
                project_qk("wk", kTp, bkc, 0, 1)
                project_qk("wq", qTp, bqc, 0, 0)
                project_qk("wq", qTp, bqc, 0, 1)
                project_v(0)
                project_v(1)
                project_qk("wk", kTp, bkc, 1, 0)
                project_qk("wk", kTp, bkc, 1, 1)
                project_qk("wq", qTp, bqc, 1, 0)
                project_qk("wq", qTp, bqc, 1, 1)
                project_v(2)
                project_v(3)
                project_qk("wk", kTp, bkc, 2, 0)
                project_qk("wk", kTp, bkc, 2, 1)
                project_v(4)
                project_v(5)
                project_qk("wk", kTp, bkc, 3, 0)
                project_qk("wk", kTp, bkc, 3, 1)
                project_v(6)
                project_v(7)

            # ---- phase 3: attention ----
            # Per 512-query block: 16 k-tile pairs. Scores for a pair fill a
            # [128,1024] PSUM tile, one wide exp -> e4m3 pt tile, then P@V
            # (2 DoubleRow matmuls) + denominator (1 DoubleRow) — emitted two
            # pairs behind the scores so the exp latency stays off the PE's
            # critical path. Final projections of the previous block are
            # interleaved at pair slots 2 and 4.
            ones8_3 = ones8[:].rearrange("p (two e) -> p two e", two=2)
            pending_finals = []

            def emit_pv(t, pt, acc, accd):
                pt3 = pt[:].rearrange("p (two q) -> p two q", two=2)
                # v8 page t holds exactly the k-tile pair 2t,2t+1: [128,2,256]
                vkte = v8p[t][:].rearrange("p (kt e) -> p kt e", e=D)
                first, last = (t == 0), (t == 15)
                for h in range(2):
                    nc.tensor.matmul(
                        acc[:, h * 512:(h + 1) * 512],
                        vkte[:, :, h * 128:(h + 1) * 128],
                        pt3, start=first, stop=last, perf_mode=DR)
                nc.tensor.matmul(accd[:], ones8_3, pt3,
                                 start=first, stop=last, perf_mode=DR)

            for qb in range(SQ // 512 if phases >= 3 else 0):
                qsl = slice(qb * 512, (qb + 1) * 512)
                acc = psacc.tile([128, 1024], F32, tag="acc", name="acc")
                accd = psacc.tile([128, 512], F32, tag="accd", name="accd")
                pts = []
                for t in range(16):
                    sp = psmm.tile([128, 1024], F32, tag="sp", name="sp",
                                   bufs=2)
                    for half in range(2):
                        st = t * 2 + half
                        pg, off = divmod(st * 128, 512)
                        psl = sp[:, half * 512:(half + 1) * 512]
                        nc.tensor.matmul(psl, kTp[0][pg][:, off:off + 128],
                                         qTp[0][qb][:],
                                         start=True, stop=False)
                        nc.tensor.matmul(psl, kTp[1][pg][:, off:off + 128],
                                         qTp[1][qb][:],
                                         start=False, stop=True)
                    pt = pt_pool.tile([128, 1024], FP8, tag="pt", name="pt",
                                      bufs=4)
                    nc.scalar.activation(pt[:], sp[:],
                                         mybir.ActivationFunctionType.Exp,
                                         scale=SCALE, bias=ebias[:])
                    pts.append(pt)
                    if t >= 2:
                        emit_pv(t - 2, pts[t - 2], acc, accd)
                    if t in (2, 4) and pending_finals:
                        pending_finals.pop(0)()
                emit_pv(14, pts[14], acc, accd)
                emit_pv(15, pts[15], acc, accd)

                rec = ovec_pool.tile([128, 512], F32, tag="rec", name="rec")
                o = [ovec_pool.tile([128, 512], F32R, tag=f"o{e}",
                                    name=f"o{e}") for e in range(2)]
                # halves: lets the first final matmuls start ~0.8us earlier
                for hsl in (slice(0, 256), slice(256, 512)):
                    nc.vector.reciprocal(rec[:, hsl], accd[:, hsl])
                    for e in range(2):
                        nc.vector.tensor_mul(
                            o[e][:, hsl], acc[:, e * 512:(e + 1) * 512][:, hsl],
                            rec[:, hsl])

                # Final projection: two row-tiles per [128,512] staging tile,
                # one paired 256-row output DMA. Deferred into the next
                # block's score stream; the last block flushes immediately,
                # using the now-free score-pool banks (one [128,1024] tile
                # per pair = two independent accumulation regions) so the
                # four units don't serialize on the single "fp" bank.
                def make_final(qb, o, pair, flush=False):
                    def emit():
                        fo = fout_pool.tile([128, 2 * D], F32, tag="fout",
                                            name="fout")
                        fpw = (psmm.tile([128, 1024], F32, tag="sp",
                                         name="sp", bufs=2) if flush else None)
                        for half in range(2):
                            t4 = pair * 2 + half
                            tsl = slice(t4 * 128, (t4 + 1) * 128)
                            fp = (fpw[:, half * 512:half * 512 + D]
                                  if flush else
                                  psmm.tile([128, D], F32, tag="fp",
                                            name="fp", bufs=1)[:])
                            for e in range(2):
                                nc.tensor.matmul(fp, _r(o[e][:, tsl]),
                                                 _r(wchunk("wo", e)),
                                                 start=(e == 0), stop=(e == 1))
                            nc.vector.tensor_add(
                                fo[:, half * D:(half + 1) * D],
                                fp, bob[:, 0:D])
                        # The two flush DMAs go out on DIFFERENT HWDGE rings
                        # (ACT + SP) so the kernel's last two transfers don't
                        # serialize on one ring's completion chaining.
                        eng = nc.scalar if (flush and pair == 0) else nc.sync
                        eng.dma_start(out_g[qb * 2 + pair],
                                      fo.rearrange("p (j c) -> p j c", j=2))
                    return emit

                if qb == SQ // 512 - 1:
                    for pair in range(2):
                        make_final(qb, o, pair, flush=True)()
                else:
                    for pair in range(2):
                        pending_finals.append(make_final(qb, o, pair))

    nc.compile()
    return nc



_NC = None


def _get_nc():
    global _NC
    if _NC is None:
        _NC = _build()
    return _NC


class _Runner:
    """Cached jitted SPMD executor (run_bass_kernel_spmd rebuilds its jax
    closure every call, forcing a retrace; this traces once)."""

    def __init__(self, nc):
        import jax
        from jax.sharding import Mesh, PartitionSpec
        from jax.experimental.shard_map import shard_map
        from concourse import bass2jax, mybir as mb

        bass2jax.install_neuronx_cc_hook()
        self.jax = jax
        if not any("axon" in str(getattr(d, "platform", "")).lower()
                   or str(d).startswith("NC_")
                   for d in jax.devices()):
            # jax was initialized on another platform (e.g. cpu for the
            # reference); reset so the axon NeuronCores are visible.
            import jax._src.xla_bridge as xb
            jax.config.update("jax_platforms", None)
            xb._clear_backends()
            if hasattr(xb.get_backend, "cache_clear"):
                xb.get_backend.cache_clear()
            if not any("axon" in str(getattr(d, "platform", "")).lower()
                       or str(d).startswith("NC_")
                       for d in jax.devices()):
                jax.config.update("jax_platforms", "axon")
                xb._clear_backends()
                if hasattr(xb.get_backend, "cache_clear"):
                    xb.get_backend.cache_clear()
        partition_name = (nc.partition_id_tensor.name
                          if nc.partition_id_tensor else None)
        in_names, out_names, out_avals = [], [], []
        for alloc in nc.m.functions[0].allocations:
            if not isinstance(alloc, mb.MemoryLocationSet):
                continue
            name = alloc.memorylocations[0].name
            if alloc.kind == "ExternalInput":
                if name != partition_name:
                    in_names.append(name)
            elif alloc.kind == "ExternalOutput":
                out_names.append(name)
                out_avals.append(jax.core.ShapedArray(
                    tuple(alloc.tensor_shape), mb.dt.np(alloc.dtype)))
        self.in_names, self.out_names, self.out_avals = \
            in_names, out_names, out_avals
        n_params, n_outs = len(in_names), len(out_names)
        bind_in_names = in_names + out_names + (
            [partition_name] if partition_name else [])

        def _body(*args):
            operands = list(args)
            if partition_name is not None:
                operands.append(bass2jax.partition_id_tensor())
            outs = bass2jax._bass_exec_p.bind(
                *operands,
                out_avals=tuple(out_avals),
                in_names=tuple(bind_in_names),
                out_names=tuple(out_names),
                lowering_input_output_aliases=(),
                sim_require_finite=True,
                sim_require_nnan=True,
                nc=nc,
            )
            return tuple(outs)

        devices = jax.devices()[:NCORES]
        mesh = Mesh(np.asarray(devices), ("core",))
        spec = (PartitionSpec("core"),) * (n_params + n_outs)
        self.fn = jax.jit(
            shard_map(_body, mesh=mesh, in_specs=spec,
                      out_specs=(PartitionSpec("core"),) * n_outs,
                      check_rep=False),
            donate_argnums=tuple(range(n_params, n_params + n_outs)),
            keep_unused=True,
        )

    def run(self, in_maps):
        concat_in = [
            np.concatenate([np.asarray(m[n]) for m in in_maps], axis=0)
            for n in self.in_names
        ]
        concat_zeros = [
            np.zeros((NCORES * a.shape[0], *a.shape[1:]), a.dtype)
            for a in self.out_avals
        ]
        outs = self.fn(*concat_in, *concat_zeros)
        return [
            {n: np.asarray(outs[i]).reshape(NCORES, *self.out_avals[i].shape)[c]
             for i, n in enumerate(self.out_names)}
            for c in range(NCORES)
        ]


_RUNNER = None


def _get_runner():
    global _RUNNER
    if _RUNNER is None:
        _RUNNER = _Runner(_get_nc())
    return _RUNNER


def kernel(**inputs):
    import ml_dtypes
    bf16 = ml_dtypes.bfloat16
    x = np.ascontiguousarray(np.asarray(inputs["x"], dtype=np.float32))
    Wq = np.asarray(inputs["Wq"], dtype=np.float32)
    Wk = np.asarray(inputs["Wk"], dtype=np.float32)
    Wv = np.asarray(inputs["Wv"], dtype=np.float32)
    Wo = np.ascontiguousarray(np.asarray(inputs["Wo"], dtype=np.float32))
    bq = np.asarray(inputs["bq"], dtype=np.float32)
    bk = np.asarray(inputs["bk"], dtype=np.float32)
    bv = np.ascontiguousarray(np.asarray(inputs["bv"], dtype=np.float32))
    bo = np.ascontiguousarray(np.asarray(inputs["bo"], dtype=np.float32))

    try:
        runner = _get_runner()
    except Exception:
        runner = None
    # bv folds into bo: attention rows sum to 1, so attn@(v+bv) = attn@v + bv.
    bo_eff = (bv @ Wo + bo).astype(np.float32)
    wqkv = np.ascontiguousarray(
        np.concatenate([Wq, Wk, Wv], axis=0).astype(bf16))
    bqk = np.ascontiguousarray(np.concatenate([bq, bk]).astype(np.float32))
    in_maps = []
    for c in range(NCORES):
        b, h = divmod(c, 2)
        # Rotate the batch so this core's queries are rows 0..SQ-1; keys and
        # values see all rows either way (softmax is key-order invariant).
        xb = (x[b] if h == 0 else
              np.concatenate([x[b, SQ:], x[b, :SQ]]))
        xbT = xb.T.astype(bf16)  # [256, 4096]
        # Pack [128, 2*S]: partition p = [chunk0 row p | chunk1 row p].
        xbTp = np.ascontiguousarray(
            np.concatenate([xbT[:128], xbT[128:]], axis=1))
        in_maps.append({
            "xkvT": xbTp, "wqkv": wqkv, "wo": Wo, "bqk": bqk, "bo": bo_eff,
        })
    results = None
    if runner is not None:
        try:
            results = runner.run(in_maps)
        except Exception:
            results = None
    if results is None:
        results = run_bass_kernel_spmd(
            _get_nc(), in_maps, core_ids=list(range(NCORES))).results
    outp = np.empty((B, S, D), dtype=np.float32)
    for c in range(NCORES):
        b, h = divmod(c, 2)
        outp[b, h * SQ:(h + 1) * SQ] = results[c]["out"]
    return outp


# revision 89
# speedup vs baseline: 1.0051x; 1.0051x over previous
"""Trainium2 Bass kernel: single-head attention module (dense transformer).

Computes, for x [4, 4096, 256] (f32) and per-projection weights/biases:
    q = x @ Wq + bq;  k = x @ Wk + bk;  v = x @ Wv + bv
    out = softmax((q k^T) / sqrt(256)) @ v @ Wo + bo

Sharding over 8 NeuronCores: core c handles batch c//2, query half c%2.
The host rotates each core's batch so its queries are always rows 0..2047
(softmax is key-order invariant), keeping the device program identical
across cores. Each core computes K/V for its whole batch (redundant with
its pair core, which is cheap) and attention + output projection for its
2048 queries.

Per-core kernel layout. Scores run in float32r (full-rate ~fp32 on the
PE, 1 col/cycle); the P@V and softmax-denominator matmuls run in
fp8-e4m3 DoubleRow mode (0.5 col/cycle, 2x PE rate) — the only
sub-f32r speedup the PE offers. Measured numerics: e4m3 P/V + bf16
x/Wqkv costs 1.49e-2 Frobenius rel err (quantizing Q/K to fp8 would
cost 2.9e-2 — over the 2e-2 budget — so scores stay f32r).
  - x^T arrives PRE-TRANSPOSED and PACKED from the host ([128, 2*S]
    bf16, both d-chunks per partition row): three wide DMAs, no PE
    transposes. wq/wk/wv are packed into one bf16 DMA (ACT's HWDGE
    ring, parallel to the x stream on SP's ring), bq/bk into another —
    the DMA queue chains issue-on-completion (~1.5us fixed per DMA in a
    ~3-deep flight window), so fewer/larger transfers shorten the
    input-critical path. ~190 dummy fp8 matmuls warm the PE's p-state
    ramp (0.65->2.4GHz over 3us continuous) under the DMA head.
  - Q^T [e, sq] / K^T [e, sk] produced directly transposed (lhsT = W
    chunk, moving = x^T). V in natural [sk, e], evicted PSUM->SBUF as
    e4m3 (the eviction converts for free). All persistent activations
    are split into [128,512]-column page tiles with exactly ONE writer
    each (the scheduler serializes cross-engine writes to a shared
    tile); evictions split per-half: DVE always h0, ACT always h1.
  - Scores for a k-tile PAIR land in one [128,1024] PSUM tile (2
    banks); ONE 1024-wide exp per pair (amortizes the ~370ns ACT fixed
    cost) writes P^T = exp(S^T/16 - 1.5) straight to an e4m3 SBUF
    tile. The -1.5 bias keeps max(P) ~ 96 < 240 (TRN e4m3 saturates to
    Inf above 240) and cancels exactly in the normalization. A dummy
    exp at t~1us pins the exp_and_others ACT table (identity/copy/exp
    share it) so no 1.3us table reload lands mid-stream.
  - P@V: per pair, two DoubleRow matmuls (e-halves) with stationary
    v8[k,2,e] and moving pt[k,2,q] accumulate out^T[e, 512q] over 16
    pairs; denominator: one DoubleRow matmul with an e4m3 ones
    stationary into accd. PV+denom are emitted TWO pairs behind scores
    so the ~1.4us exp release never stalls the PE. PSUM = 2x2 (scores)
    + 2 (acc) + 1 (accd) + 1 (final proj) = 8 banks exactly.
  - out^T is scaled by 1/denom (DVE) and fed as the stationary of the
    final f32r projection, landing output in natural [sq, f] layout
    for paired 256-row output DMAs. Final projections of block qb are
    interleaved into block qb+1's score stream; the last block flushes
    through the freed score banks with its two out-DMAs on different
    HWDGE rings. bo broadcasts via GPSIMD partition_broadcast (a PE
    ones-matmul would let the scheduler gate attention on the late bo
    DMA); bv folds into bo host-side (attention rows sum to 1).

Sim/HW exec: 123644 ns/core (baseline 169150; PE ~102us busy of which
~96us is real work: scores 54.6 + PV/denom 20.5 + projections 17.1 +
finals 3.4; ACT exp 66; the ~21us of PE idle is the DMA-bound head,
~120ns/pair exp-release slack in steady state, and the end drain).
"""

import numpy as np

import concourse.bass as bass  # noqa: F401  (AP types come through tile/bacc)
import concourse.tile as tile
from concourse import bacc, mybir
from concourse.bass_utils import run_bass_kernel_spmd

B, S, D = 4, 4096, 256
SQ = S // 2  # queries per core
NCORES = 8
F32 = mybir.dt.float32
F32R = mybir.dt.float32r
BF16 = mybir.dt.bfloat16
FP8 = mybir.dt.float8e4
U8 = mybir.dt.uint8
SCALE = 1.0 / 16.0  # 1/sqrt(D)
EXP_BIAS = -1.5  # exp(s/16 - 1.5): max scaled score ~6.1 -> max P ~ e^4.6=99
DR = mybir.MatmulPerfMode.DoubleRow


def _r(ap):
    """View an fp32 AP as float32r: full-rate fp32 matmul on the PE."""
    return ap.bitcast(F32R)


def _build(phases=3):
    nc = bacc.Bacc("TRN2", target_bir_lowering=False, debug=False,
                   num_devices=NCORES)

    # x and the packed Q/K/V weights arrive as bf16 (host converts): bf16
    # enables the XBAR DMA-transpose of x (2-byte dtypes only), halves the x
    # DMA traffic, and costs ~1e-3 rel err against the 2e-2 budget. Wo stays
    # f32 (its matmul partner o is f32r). wq/wk/wv are packed into ONE DRAM
    # tensor (and bq/bk likewise) because each dma_start costs ~650ns on the
    # sequencer + HWDGE AND the DMA queue chains issue on completion (~1.5us
    # fixed per DMA): fewer, larger DMAs shorten the input stream critically.
    # x arrives PRE-TRANSPOSED from the host ([D, S] bf16): x^T is what every
    # projection consumes, host transposition is free w.r.t. HW exec time,
    # and loading it with 4-6 plain wide DMAs beats 8 XBAR DMA-transposes on
    # the chained DMA queue (~1.5us fixed cost per DMA in flight-window 3).
    # Packed layout [128, 2*S]: partition p holds d-chunk0 row p then
    # d-chunk1 row p, so ONE wide DMA delivers both contraction chunks.
    xkvT_d = nc.dram_tensor("xkvT", [128, 2 * S], BF16,
                            kind="ExternalInput").ap()
    wqkv = nc.dram_tensor("wqkv", [3 * D, D], BF16, kind="ExternalInput").ap()
    wo_d = nc.dram_tensor("wo", [D, D], F32, kind="ExternalInput").ap()
    bqk = nc.dram_tensor("bqk", [2 * D], F32, kind="ExternalInput").ap()
    bo_d = nc.dram_tensor("bo", [D], F32, kind="ExternalInput").ap()
    out = nc.dram_tensor("out", [SQ, D], F32, kind="ExternalOutput").ap()

    bo_row = bo_d.rearrange("(a b) -> a b", a=1)  # [1, 256]
    bqk_pnc = bqk.rearrange("(n c p) -> p (n c)", n=2, p=128)  # [128, 4]
    wqkv_g = wqkv.rearrange("(n j p) c -> p n j c", n=3, j=2)  # [128,3,2,256]
    wo_g = wo_d.rearrange("(j p) c -> p j c", j=2)
    out_g = out.rearrange("(g j p) c -> g p j c", j=2, p=128)   # [8,128,2,256]

    with tile.TileContext(nc) as tc:
        with (
            tc.tile_pool(name="const", bufs=1) as cpool,
            tc.tile_pool(name="pt", bufs=4) as pt_pool,
            tc.tile_pool(name="ovec", bufs=2) as ovec_pool,
            tc.tile_pool(name="fout", bufs=2) as fout_pool,
            tc.tile_pool(name="psmm", bufs=1, space="PSUM") as psmm,
            tc.tile_pool(name="psacc", bufs=1, space="PSUM") as psacc,
        ):
            # ---- constants ----
            # e4m3 ones [128, 2*128] for the DoubleRow denominator matmul
            ones8 = cpool.tile([128, 256], FP8, tag="ones8", name="ones8")
            # memset on GPSIMD: lands ~0.5us earlier than DVE (it gates the
            # PE warm-up stream below).
            nc.gpsimd.memset(ones8[:].bitcast(U8), 0x38)  # e4m3 1.0
            ebias = cpool.tile([128, 1], F32, tag="ebias", name="ebias")
            nc.vector.memset(ebias[:], EXP_BIAS)
            # Dummy exp pins the exp_and_others ACT table now (~t=1us, during
            # the DMA head); identity/copy/exp all live in that set, so no
            # 1.3us table reload ever lands in front of the attention exps.
            scratch1 = cpool.tile([128, 1], F32, tag="scr1", name="scr1")
            nc.scalar.activation(scratch1[:], ebias[:],
                                 mybir.ActivationFunctionType.Exp)

            # ---- persistent activations, split into [128,512]-column pages
            # so every eviction writes exactly one page (single writer per
            # tile: the scheduler serializes cross-engine writes to a shared
            # tile, which would otherwise convoy the DVE/ACT eviction pairs).
            def pages(tag, n, dt=F32R):
                return [cpool.tile([128, 512], dt, tag=f"{tag}_{p}",
                                   name=f"{tag}_{p}") for p in range(n)]

            # x^T lives in three packed tiles (s-ranges 0:1024, 1024:2048,
            # 2048:4096; each holds both d-chunks side by side, matching the
            # packed DRAM layout): one DMA per tile, single writer. The
            # 0.5/0.5/1 MiB split gets K0/Q0 going ~1.5us earlier than two
            # 1 MiB slabs would while keeping the chained-DMA count low.
            XRANGES = [(0, 1024), (1024, 2048), (2048, 4096)]
            xkvTt = [cpool.tile([128, 2 * (b - a)], BF16, tag=f"xkvT_{i}",
                                name=f"xkvT_{i}")
                     for i, (a, b) in enumerate(XRANGES)]
            qTp = [pages(f"qT{c}", 4) for c in range(2)]
            kTp = [pages(f"kT{c}", 8) for c in range(2)]
            v8p = pages("v8", 16, dt=FP8)

            # wq/wk/wv in one packed bf16 tile [128, 3*2*256]; wo f32r.
            wqkv_sb = cpool.tile([128, 6 * D], BF16, tag="wqkv", name="wqkv")
            wo_sb = cpool.tile([128, 2 * D], F32R, tag="w_wo", name="w_wo")
            _widx = {"wq": 0, "wk": 1, "wv": 2}

            def wchunk(n, c):  # [128, 256] d-chunk c of W
                if n == "wo":
                    return wo_sb[:, c * D:(c + 1) * D]
                return wqkv_sb[:, (_widx[n] * 2 + c) * D:
                               (_widx[n] * 2 + c + 1) * D]

            # Packed biases: [128, 4] = (bq c0, bq c1, bk c0, bk c1).
            b4 = cpool.tile([128, 4], F32, tag="b4", name="b4")
            bqc = [b4[:, c:c + 1] for c in range(2)]
            bkc = [b4[:, 2 + c:3 + c] for c in range(2)]

            # ---- DMA issue order = transfer order (single serial HWDGE +
            # DMA-engine chain, ~3 DMAs in flight globally).
            xkvT_cs = xkvT_d.rearrange("p (c s) -> p c s", c=2)

            def dma_xT(i):
                a, b = XRANGES[i]
                nc.sync.dma_start(
                    xkvTt[i].rearrange("p (c s) -> p c s", c=2),
                    xkvT_cs[:, :, a:b])

            # Weights/biases go out on the ACT sequencer's HWDGE ring (TRN2
            # has two physical rings: qSPDynamicHW + qActDynamicHW), so their
            # issue chain runs in parallel with the x^T stream on SP.
            nc.scalar.dma_start(
                wqkv_sb.rearrange("p (n j c) -> p n j c", n=3, j=2),
                wqkv_g[:])
            nc.scalar.dma_start(b4[:], bqk_pnc)
            dma_xT(0)
            dma_xT(1)
            dma_xT(2)
            nc.sync.dma_start(
                wo_sb.rearrange("p (j c) -> p j c", j=2), _r(wo_g[:]))

            # bo broadcast across partitions on the (idle) GPSIMD engine:
            # row DMA [1,256] then partition-broadcast into both halves of
            # bob [128,512], so one wide add covers two output row-tiles.
            # No PE involvement — an fp32 ones-matmul here would let the
            # scheduler gate the attention stream on this late DMA. (bv is
            # folded into bo host-side: attention rows sum to 1.)
            bob = cpool.tile([128, 2 * D], F32, tag="bob", name="bob")
            row = cpool.tile([1, D], F32, tag="bor", name="bor")
            nc.sync.dma_start(row[:], bo_row[:])
            for half in range(2):
                nc.gpsimd.partition_broadcast(
                    bob[:, half * D:(half + 1) * D], row[:])

            # PE p-state warm-up: ~170 dummy DoubleRow matmuls on the ones8
            # tile keep the PE continuously busy from ~1.3us (after the ones8
            # memset) until the first x^T slab + weights land (~6us). The PE
            # clock ramps 0.65 -> 1.2 -> 2.4 GHz over 3us of CONTINUOUS
            # execution and resets on idle, so without this the whole first
            # ~3us of projections would run at half clock.
            ones8_3 = ones8[:].rearrange("p (two e) -> p two e", two=2)
            warm = psmm.tile([128, D], F32, tag="fp", name="fp", bufs=1)
            for _ in range(190):
                nc.tensor.matmul(warm[:, 0:128], ones8_3, ones8_3,
                                 start=True, stop=True, perf_mode=DR)

            # ---- phases 1+2, interleaved per x group ----
            # Each [128,1024] PSUM tile is processed as two independent
            # halves: matmuls for half h, then its eviction — half 0 always
            # on DVE, half 1 always on ACT, each into its own page tile. The
            # eviction of half 0 completes while the PE fills half 1, so the
            # 2-deep PSUM ring never stalls the PE.
            def evict1(dsl, src, h, bias=None):
                if bias is None:
                    if h == 0:
                        nc.vector.tensor_copy(dsl, src)
                    else:
                        nc.scalar.copy(dsl, src)
                elif h == 0:
                    nc.vector.tensor_scalar_add(dsl, src, bias)
                else:
                    nc.scalar.activation(
                        dsl, src, mybir.ActivationFunctionType.Identity,
                        bias=bias)

            def xT(c, col, width=512):  # read helper over x^T slabs
                for t, (a, b) in enumerate(XRANGES):
                    if col < b:
                        off = col - a
                        w = b - a
                        return xkvTt[t][:, c * w + off:c * w + off + width]

            def project_qk(wn, dstTp, bcol, g, et):
                # lhsT = W[d_chunk, e_tile], moving = x^T[d_chunk, group g].
                pp = psmm.tile([128, 1024], F32, tag="sp", name="sp",
                               bufs=2)
                for h in range(2):
                    blk = g * 2 + h
                    for c in range(2):
                        nc.tensor.matmul(
                            pp[:, h * 512:(h + 1) * 512],
                            wchunk(wn, c)[:, et * 128:(et + 1) * 128],
                            xT(c, blk * 512),
                            start=(c == 0), stop=(c == 1),
                        )
                    evict1(dstTp[et][blk][:],
                           pp[:, h * 512:(h + 1) * 512],
                           h, bias=bcol[et])

            def project_v(stp):
                # V natural layout [sk, e]; four sk-tiles per [128,1024]
                # PSUM tile; each half's eviction converts fp32 -> e4m3 into
                # one v8 page (= one k-tile pair, the PV stationary unit).
                vp = psmm.tile([128, 1024], F32, tag="sp", name="sp", bufs=2)
                for h in range(2):
                    for quad in (h * 2, h * 2 + 1):
                        st = stp * 4 + quad
                        for c in range(2):
                            nc.tensor.matmul(
                                vp[:, quad * D:(quad + 1) * D],
                                xT(c, st * 128, 128),
                                wchunk("wv", c),
                                start=(c == 0), stop=(c == 1),
                            )
                    evict1(v8p[stp * 2 + h][:],
                           vp[:, h * 512:(h + 1) * 512], h)

            # Strictly slab-major: all tiles reading x^T slab i are emitted
            # together, in DMA arrival order, so the PE consumes each slab
            # at the rate the chained DMA queue delivers them. (Interleaving
            # projections into the score stream was tried and loses: with the
            # 2-deep PSUM ring, consecutive score tiles end up 2 allocations
            # apart and an interposed projection tile provides less cover
            # than attention's own S+2xPV stream for the ~1.4us exp release.)
            if phases >= 2:
                project_qk("wk", kTp, bkc, 0, 0)
                project_qk("wk", kTp, bkc, 0, 1)
                project_qk("wq", qTp, bqc, 0, 0)
                project_qk("wq", qTp, bqc, 0, 1)
                project_v(0)
                project_v(1)
                project_qk("wk", kTp, bkc, 1, 0)
                project_qk("wk", kTp, bkc, 1, 1)
                project_qk("wq", qTp, bqc, 1, 0)
                project_qk("wq", qTp, bqc, 1, 1)
                project_v(2)
                project_v(3)
                project_qk("wk", kTp, bkc, 2, 0)
                project_qk("wk", kTp, bkc, 2, 1)
                project_v(4)
                project_v(5)
                project_qk("wk", kTp, bkc, 3, 0)
                project_qk("wk", kTp, bkc, 3, 1)
                project_v(6)
                project_v(7)

            # ---- phase 3: attention ----
            # Per 512-query block: 16 k-tile pairs. Scores for a pair fill a
            # [128,1024] PSUM tile, one wide exp -> e4m3 pt tile, then P@V
            # (2 DoubleRow matmuls) + denominator (1 DoubleRow) — emitted two
            # pairs behind the scores so the exp latency stays off the PE's
            # critical path. Final projections of the previous block are
            # interleaved at pair slots 2 and 4.
            ones8_3 = ones8[:].rearrange("p (two e) -> p two e", two=2)
            pending_finals = []

            def emit_pv(t, pt, acc, accd):
                pt3 = pt[:].rearrange("p (two q) -> p two q", two=2)
                # v8 page t holds exactly the k-tile pair 2t,2t+1: [128,2,256]
                vkte = v8p[t][:].rearrange("p (kt e) -> p kt e", e=D)
                first, last = (t == 0), (t == 15)
                for h in range(2):
                    nc.tensor.matmul(
                        acc[:, h * 512:(h + 1) * 512],
                        vkte[:, :, h * 128:(h + 1) * 128],
                        pt3, start=first, stop=last, perf_mode=DR)
                nc.tensor.matmul(accd[:], ones8_3, pt3,
                                 start=first, stop=last, perf_mode=DR)

            for qb in range(SQ // 512 if phases >= 3 else 0):
                qsl = slice(qb * 512, (qb + 1) * 512)
                acc = psacc.tile([128, 1024], F32, tag="acc", name="acc")
                accd = psacc.tile([128, 512], F32, tag="accd", name="accd")
                pts = []
                for t in range(16):
                    sp = psmm.tile([128, 1024], F32, tag="sp", name="sp",
                                   bufs=2)
                    for half in range(2):
                        st = t * 2 + half
                        pg, off = divmod(st * 128, 512)
                        psl = sp[:, half * 512:(half + 1) * 512]
                        nc.tensor.matmul(psl, kTp[0][pg][:, off:off + 128],
                                         qTp[0][qb][:],
                                         start=True, stop=False)
                        nc.tensor.matmul(psl, kTp[1][pg][:, off:off + 128],
                                         qTp[1][qb][:],
                                         start=False, stop=True)
                    pt = pt_pool.tile([128, 1024], FP8, tag="pt", name="pt",
                                      bufs=4)
                    nc.scalar.activation(pt[:], sp[:],
                                         mybir.ActivationFunctionType.Exp,
                                         scale=SCALE, bias=ebias[:])
                    pts.append(pt)
                    if t >= 2:
                        emit_pv(t - 2, pts[t - 2], acc, accd)
                    # Slot 15's final pads the PE between S15 and the next
                    # block's S0 (which must wait exp15's PSUM-ring release,
                    # ~1.4us after S15 — the three PV emissions alone leave
                    # a ~460ns hole). Query-block 0 has no finals yet, and
                    # the last block's tail waits on exp15 regardless: pad
                    # with dummy fp8 matmuls so the p-state ramp never
                    # resets there.
                    if t in (2, 15) and pending_finals:
                        pending_finals.pop(0)()
                    elif t == 15 and qb == 0:
                        wt = psmm.tile([128, D], F32, tag="fp", name="fp",
                                       bufs=1)
                        for _ in range(16):
                            nc.tensor.matmul(wt[:, 0:128], ones8_3, ones8_3,
                                             start=True, stop=True,
                                             perf_mode=DR)
                    if t == 15 and qb == SQ // 512 - 1:
                        wt = psmm.tile([128, D], F32, tag="fp", name="fp",
                                       bufs=1)
                        for _ in range(36):
                            nc.tensor.matmul(wt[:, 0:128], ones8_3, ones8_3,
                                             start=True, stop=True,
                                             perf_mode=DR)
                emit_pv(14, pts[14], acc, accd)
                emit_pv(15, pts[15], acc, accd)

                rec = ovec_pool.tile([128, 512], F32, tag="rec", name="rec")
                o = [ovec_pool.tile([128, 512], F32R, tag=f"o{e}",
                                    name=f"o{e}") for e in range(2)]
                # halves: lets the first final matmuls start ~0.8us earlier
                for hsl in (slice(0, 256), slice(256, 512)):
                    nc.vector.reciprocal(rec[:, hsl], accd[:, hsl])
                    for e in range(2):
                        nc.vector.tensor_mul(
                            o[e][:, hsl], acc[:, e * 512:(e + 1) * 512][:, hsl],
                            rec[:, hsl])

                # Final projection: two row-tiles per [128,512] staging tile,
                # one paired 256-row output DMA. Deferred into the next
                # block's score stream; the last block flushes immediately,
                # using the now-free score-pool banks (one [128,1024] tile
                # per pair = two independent accumulation regions) so the
                # four units don't serialize on the single "fp" bank.
                def make_final(qb, o, pair, flush=False):
                    def emit():
                        fo = fout_pool.tile([128, 2 * D], F32, tag="fout",
                                            name="fout")
                        # Flush: both row-tiles of the pair land in adjacent
                        # [128,256] regions of one bank, so ONE wide add
                        # covers the pair (one fewer DVE op on the tail's
                        # serial chain).
                        fpw = (psmm.tile([128, 1024], F32, tag="sp",
                                         name="sp", bufs=2) if flush else None)
                        for half in range(2):
                            t4 = pair * 2 + half
                            tsl = slice(t4 * 128, (t4 + 1) * 128)
                            fp = (fpw[:, half * D:(half + 1) * D]
                                  if flush else
                                  psmm.tile([128, D], F32, tag="fp",
                                            name="fp", bufs=1)[:])
                            for e in range(2):
                                nc.tensor.matmul(fp, _r(o[e][:, tsl]),
                                                 _r(wchunk("wo", e)),
                                                 start=(e == 0), stop=(e == 1))
                            if not flush:
                                nc.vector.tensor_add(
                                    fo[:, half * D:(half + 1) * D],
                                    fp, bob[:, 0:D])
                        if flush:
                            nc.vector.tensor_add(fo[:], fpw[:, 0:2 * D],
                                                 bob[:])
                        # The two flush DMAs go out on DIFFERENT HWDGE rings
                        # (ACT + SP) so the kernel's last two transfers don't
                        # serialize on one ring's completion chaining.
                        eng = nc.scalar if (flush and pair == 0) else nc.sync
                        eng.dma_start(out_g[qb * 2 + pair],
                                      fo.rearrange("p (j c) -> p j c", j=2))
                    return emit

                if qb == SQ // 512 - 1:
                    for pair in range(2):
                        make_final(qb, o, pair, flush=True)()
                else:
                    for pair in range(2):
                        pending_finals.append(make_final(qb, o, pair))

    nc.compile()
    return nc



_NC = None


def _get_nc():
    global _NC
    if _NC is None:
        _NC = _build()
    return _NC


class _Runner:
    """Cached jitted SPMD executor (run_bass_kernel_spmd rebuilds its jax
    closure every call, forcing a retrace; this traces once)."""

    def __init__(self, nc):
        import jax
        from jax.sharding import Mesh, PartitionSpec
        from jax.experimental.shard_map import shard_map
        from concourse import bass2jax, mybir as mb

        bass2jax.install_neuronx_cc_hook()
        self.jax = jax
        if not any("axon" in str(getattr(d, "platform", "")).lower()
                   or str(d).startswith("NC_")
                   for d in jax.devices()):
            # jax was initialized on another platform (e.g. cpu for the
            # reference); reset so the axon NeuronCores are visible.
            import jax._src.xla_bridge as xb
            jax.config.update("jax_platforms", None)
            xb._clear_backends()
            if hasattr(xb.get_backend, "cache_clear"):
                xb.get_backend.cache_clear()
            if not any("axon" in str(getattr(d, "platform", "")).lower()
                       or str(d).startswith("NC_")
                       for d in jax.devices()):
                jax.config.update("jax_platforms", "axon")
                xb._clear_backends()
                if hasattr(xb.get_backend, "cache_clear"):
                    xb.get_backend.cache_clear()
        partition_name = (nc.partition_id_tensor.name
                          if nc.partition_id_tensor else None)
        in_names, out_names, out_avals = [], [], []
        for alloc in nc.m.functions[0].allocations:
            if not isinstance(alloc, mb.MemoryLocationSet):
                continue
            name = alloc.memorylocations[0].name
            if alloc.kind == "ExternalInput":
                if name != partition_name:
                    in_names.append(name)
            elif alloc.kind == "ExternalOutput":
                out_names.append(name)
                out_avals.append(jax.core.ShapedArray(
                    tuple(alloc.tensor_shape), mb.dt.np(alloc.dtype)))
        self.in_names, self.out_names, self.out_avals = \
            in_names, out_names, out_avals
        n_params, n_outs = len(in_names), len(out_names)
        bind_in_names = in_names + out_names + (
            [partition_name] if partition_name else [])

        def _body(*args):
            operands = list(args)
            if partition_name is not None:
                operands.append(bass2jax.partition_id_tensor())
            outs = bass2jax._bass_exec_p.bind(
                *operands,
                out_avals=tuple(out_avals),
                in_names=tuple(bind_in_names),
                out_names=tuple(out_names),
                lowering_input_output_aliases=(),
                sim_require_finite=True,
                sim_require_nnan=True,
                nc=nc,
            )
            return tuple(outs)

        devices = jax.devices()[:NCORES]
        mesh = Mesh(np.asarray(devices), ("core",))
        spec = (PartitionSpec("core"),) * (n_params + n_outs)
        self.fn = jax.jit(
            shard_map(_body, mesh=mesh, in_specs=spec,
                      out_specs=(PartitionSpec("core"),) * n_outs,
                      check_rep=False),
            donate_argnums=tuple(range(n_params, n_params + n_outs)),
            keep_unused=True,
        )

    def run(self, in_maps):
        concat_in = [
            np.concatenate([np.asarray(m[n]) for m in in_maps], axis=0)
            for n in self.in_names
        ]
        concat_zeros = [
            np.zeros((NCORES * a.shape[0], *a.shape[1:]), a.dtype)
            for a in self.out_avals
        ]
        outs = self.fn(*concat_in, *concat_zeros)
        return [
            {n: np.asarray(outs[i]).reshape(NCORES, *self.out_avals[i].shape)[c]
             for i, n in enumerate(self.out_names)}
            for c in range(NCORES)
        ]


_RUNNER = None


def _get_runner():
    global _RUNNER
    if _RUNNER is None:
        _RUNNER = _Runner(_get_nc())
    return _RUNNER


def kernel(**inputs):
    import ml_dtypes
    bf16 = ml_dtypes.bfloat16
    x = np.ascontiguousarray(np.asarray(inputs["x"], dtype=np.float32))
    Wq = np.asarray(inputs["Wq"], dtype=np.float32)
    Wk = np.asarray(inputs["Wk"], dtype=np.float32)
    Wv = np.asarray(inputs["Wv"], dtype=np.float32)
    Wo = np.ascontiguousarray(np.asarray(inputs["Wo"], dtype=np.float32))
    bq = np.asarray(inputs["bq"], dtype=np.float32)
    bk = np.asarray(inputs["bk"], dtype=np.float32)
    bv = np.ascontiguousarray(np.asarray(inputs["bv"], dtype=np.float32))
    bo = np.ascontiguousarray(np.asarray(inputs["bo"], dtype=np.float32))

    try:
        runner = _get_runner()
    except Exception:
        runner = None
    # bv folds into bo: attention rows sum to 1, so attn@(v+bv) = attn@v + bv.
    bo_eff = (bv @ Wo + bo).astype(np.float32)
    wqkv = np.ascontiguousarray(
        np.concatenate([Wq, Wk, Wv], axis=0).astype(bf16))
    bqk = np.ascontiguousarray(np.concatenate([bq, bk]).astype(np.float32))
    in_maps = []
    for c in range(NCORES):
        b, h = divmod(c, 2)
        # Rotate the batch so this core's queries are rows 0..SQ-1; keys and
        # values see all rows either way (softmax is key-order invariant).
        xb = (x[b] if h == 0 else
              np.concatenate([x[b, SQ:], x[b, :SQ]]))
        xbT = xb.T.astype(bf16)  # [256, 4096]
        # Pack [128, 2*S]: partition p = [chunk0 row p | chunk1 row p].
        xbTp = np.ascontiguousarray(
            np.concatenate([xbT[:128], xbT[128:]], axis=1))
        in_maps.append({
            "xkvT": xbTp, "wqkv": wqkv, "wo": Wo, "bqk": bqk, "bo": bo_eff,
        })
    results = None
    if runner is not None:
        try:
            results = runner.run(in_maps)
        except Exception:
            results = None
    if results is None:
        results = run_bass_kernel_spmd(
            _get_nc(), in_maps, core_ids=list(range(NCORES))).results
    outp = np.empty((B, S, D), dtype=np.float32)
    for c in range(NCORES):
        b, h = divmod(c, 2)
        outp[b, h * SQ:(h + 1) * SQ] = results[c]["out"]
    return outp


# revision 101
# speedup vs baseline: 1.0244x; 1.0192x over previous
"""Trainium2 Bass kernel: single-head attention module (dense transformer).

Computes, for x [4, 4096, 256] (f32) and per-projection weights/biases:
    q = x @ Wq + bq;  k = x @ Wk + bk;  v = x @ Wv + bv
    out = softmax((q k^T) / sqrt(256)) @ v @ Wo + bo

Sharding over 8 NeuronCores: core c handles batch c//2, query half c%2.
The host rotates each core's batch so its queries are always rows 0..2047
(softmax is key-order invariant), keeping the device program identical
across cores. Each core computes K/V for its whole batch (redundant with
its pair core, which is cheap) and attention + output projection for its
2048 queries.

Per-core kernel layout. Scores run in float32r (full-rate ~fp32 on the
PE, 1 col/cycle); the P@V and softmax-denominator matmuls run in
fp8-e4m3 DoubleRow mode (0.5 col/cycle, 2x PE rate) — the only
sub-f32r speedup the PE offers. Measured numerics: e4m3 P/V + bf16
x/Wqkv costs 1.49e-2 Frobenius rel err (quantizing Q/K to fp8 would
cost 2.9e-2 — over the 2e-2 budget — so scores stay f32r).
  - x^T arrives PRE-TRANSPOSED and PACKED from the host ([128, 2*S]
    bf16, both d-chunks per partition row): four wide DMAs, no PE
    transposes. wq/wk/wv are packed into one bf16 DMA (ACT's HWDGE
    ring, parallel to the x stream on SP's ring), bq/bk into another —
    the DMA queue chains issue-on-completion (~1.5us fixed per DMA in a
    ~3-deep flight window), so fewer/larger transfers shorten the
    input-critical path. ~85 dummy fp8 matmuls warm the PE's p-state
    ramp (0.65->2.4GHz over 3us continuous) under the DMA head.
  - Q^T [e, sq] / K^T [e, sk] produced directly transposed (lhsT = W
    chunk, moving = x^T). V in natural [sk, e], evicted PSUM->SBUF as
    e4m3 (the eviction converts for free). All persistent activations
    are split into [128,512]-column page tiles with exactly ONE writer
    each (the scheduler serializes cross-engine writes to a shared
    tile); evictions split per-half: DVE always h0, ACT always h1.
  - Scores for a k-tile PAIR land in one [128,1024] PSUM tile (2
    banks); ONE 1024-wide exp per pair (amortizes the ~370ns ACT fixed
    cost) writes P^T = exp(S^T/16 - 1.5) straight to an e4m3 SBUF
    tile. The -1.5 bias keeps max(P) ~ 96 < 240 (TRN e4m3 saturates to
    Inf above 240) and cancels exactly in the normalization. A dummy
    exp at t~1us pins the exp_and_others ACT table (identity/copy/exp
    share it) so no 1.3us table reload lands mid-stream.
  - P@V: per pair, two DoubleRow matmuls (e-halves) with stationary
    v8[k,2,e] and moving pt[k,2,q] accumulate out^T[e, 512q] over 16
    pairs; denominator: one DoubleRow matmul with an e4m3 ones
    stationary into accd. PV+denom are emitted TWO pairs behind scores
    so the ~1.4us exp release never stalls the PE. PSUM = 2x2 (scores)
    + 2 (acc) + 1 (accd) + 1 (final proj) = 8 banks exactly.
  - out^T is scaled by 1/denom (DVE) and fed as the stationary of the
    final f32r projection, landing output in natural [sq, f] layout
    for paired 256-row output DMAs. Final projections of block qb are
    interleaved into block qb+1's score stream; the last block flushes
    through the freed score banks with its two out-DMAs on different
    HWDGE rings. bo broadcasts via GPSIMD partition_broadcast (a PE
    ones-matmul would let the scheduler gate attention on the late bo
    DMA); bv folds into bo host-side (attention rows sum to 1).

Sim/HW exec: 120699 ns/core (baseline 169150; PE ~102us busy of which
~96us is real work: scores 54.6 + PV/denom 20.5 + projections 17.1 +
finals 3.4; ACT exp 66; the ~21us of PE idle is the DMA-bound head,
~120ns/pair exp-release slack in steady state, and the end drain).
"""

import numpy as np

import concourse.bass as bass  # noqa: F401  (AP types come through tile/bacc)
import concourse.tile as tile
from concourse import bacc, mybir
from concourse.bass_utils import run_bass_kernel_spmd

B, S, D = 4, 4096, 256
SQ = S // 2  # queries per core
NCORES = 8
F32 = mybir.dt.float32
F32R = mybir.dt.float32r
BF16 = mybir.dt.bfloat16
FP8 = mybir.dt.float8e4
U8 = mybir.dt.uint8
SCALE = 1.0 / 16.0  # 1/sqrt(D)
EXP_BIAS = -1.5  # exp(s/16 - 1.5): max scaled score ~6.1 -> max P ~ e^4.6=99
DR = mybir.MatmulPerfMode.DoubleRow


def _r(ap):
    """View an fp32 AP as float32r: full-rate fp32 matmul on the PE."""
    return ap.bitcast(F32R)


def _build(phases=3):
    nc = bacc.Bacc("TRN2", target_bir_lowering=False, debug=False,
                   num_devices=NCORES)

    # x and the packed Q/K/V weights arrive as bf16 (host converts): bf16
    # enables the XBAR DMA-transpose of x (2-byte dtypes only), halves the x
    # DMA traffic, and costs ~1e-3 rel err against the 2e-2 budget. Wo stays
    # f32 (its matmul partner o is f32r). wq/wk/wv are packed into ONE DRAM
    # tensor (and bq/bk likewise) because each dma_start costs ~650ns on the
    # sequencer + HWDGE AND the DMA queue chains issue on completion (~1.5us
    # fixed per DMA): fewer, larger DMAs shorten the input stream critically.
    # x arrives PRE-TRANSPOSED from the host ([D, S] bf16): x^T is what every
    # projection consumes, host transposition is free w.r.t. HW exec time,
    # and loading it with 4-6 plain wide DMAs beats 8 XBAR DMA-transposes on
    # the chained DMA queue (~1.5us fixed cost per DMA in flight-window 3).
    # Packed layout [128, 2*S]: partition p holds d-chunk0 row p then
    # d-chunk1 row p, so ONE wide DMA delivers both contraction chunks.
    xkvT_d = nc.dram_tensor("xkvT", [128, 2 * S], BF16,
                            kind="ExternalInput").ap()
    wqkv = nc.dram_tensor("wqkv", [3 * D, D], BF16, kind="ExternalInput").ap()
    wo_d = nc.dram_tensor("wo", [D, D], F32, kind="ExternalInput").ap()
    bqk = nc.dram_tensor("bqk", [2 * D], F32, kind="ExternalInput").ap()
    bo_d = nc.dram_tensor("bo", [D], F32, kind="ExternalInput").ap()
    out = nc.dram_tensor("out", [SQ, D], F32, kind="ExternalOutput").ap()

    bo_row = bo_d.rearrange("(a b) -> a b", a=1)  # [1, 256]
    bqk_pnc = bqk.rearrange("(n c p) -> p (n c)", n=2, p=128)  # [128, 4]
    wqkv_g = wqkv.rearrange("(n j p) c -> p n j c", n=3, j=2)  # [128,3,2,256]
    wo_g = wo_d.rearrange("(j p) c -> p j c", j=2)
    out_g = out.rearrange("(g j p) c -> g p j c", j=2, p=128)   # [8,128,2,256]

    with tile.TileContext(nc) as tc:
        with (
            tc.tile_pool(name="const", bufs=1) as cpool,
            tc.tile_pool(name="pt", bufs=4) as pt_pool,
            tc.tile_pool(name="ovec", bufs=2) as ovec_pool,
            tc.tile_pool(name="fout", bufs=2) as fout_pool,
            tc.tile_pool(name="psmm", bufs=1, space="PSUM") as psmm,
            tc.tile_pool(name="psacc", bufs=1, space="PSUM") as psacc,
        ):
            # ---- constants ----
            # e4m3 ones [128, 2*128] for the DoubleRow denominator matmul
            ones8 = cpool.tile([128, 256], FP8, tag="ones8", name="ones8")
            # memset on GPSIMD: lands ~0.5us earlier than DVE (it gates the
            # PE warm-up stream below).
            nc.gpsimd.memset(ones8[:].bitcast(U8), 0x38)  # e4m3 1.0
            ebias = cpool.tile([128, 1], F32, tag="ebias", name="ebias")
            nc.vector.memset(ebias[:], EXP_BIAS)
            # Dummy exp pins the exp_and_others ACT table now (~t=1us, during
            # the DMA head); identity/copy/exp all live in that set, so no
            # 1.3us table reload ever lands in front of the attention exps.
            scratch1 = cpool.tile([128, 1], F32, tag="scr1", name="scr1")
            nc.scalar.activation(scratch1[:], ebias[:],
                                 mybir.ActivationFunctionType.Exp)

            # ---- persistent activations, split into [128,512]-column pages
            # so every eviction writes exactly one page (single writer per
            # tile: the scheduler serializes cross-engine writes to a shared
            # tile, which would otherwise convoy the DVE/ACT eviction pairs).
            def pages(tag, n, dt=F32R):
                return [cpool.tile([128, 512], dt, tag=f"{tag}_{p}",
                                   name=f"{tag}_{p}") for p in range(n)]

            # x^T lives in three packed tiles (s-ranges 0:1024, 1024:2048,
            # 2048:4096; each holds both d-chunks side by side, matching the
            # packed DRAM layout): one DMA per tile, single writer. The
            # 0.5/0.5/1 MiB split gets K0/Q0 going ~1.5us earlier than two
            # 1 MiB slabs would while keeping the chained-DMA count low.
            XRANGES = [(0, 512), (512, 1024), (1024, 2048), (2048, 4096)]
            xkvTt = [cpool.tile([128, 2 * (b - a)], BF16, tag=f"xkvT_{i}",
                                name=f"xkvT_{i}")
                     for i, (a, b) in enumerate(XRANGES)]
            qTp = [pages(f"qT{c}", 4) for c in range(2)]
            kTp = [pages(f"kT{c}", 8) for c in range(2)]
            v8p = pages("v8", 16, dt=FP8)

            # wq/wk/wv in one packed bf16 tile [128, 3*2*256]; wo f32r.
            wqkv_sb = cpool.tile([128, 6 * D], BF16, tag="wqkv", name="wqkv")
            wo_sb = cpool.tile([128, 2 * D], F32R, tag="w_wo", name="w_wo")
            _widx = {"wq": 0, "wk": 1, "wv": 2}

            def wchunk(n, c):  # [128, 256] d-chunk c of W
                if n == "wo":
                    return wo_sb[:, c * D:(c + 1) * D]
                return wqkv_sb[:, (_widx[n] * 2 + c) * D:
                               (_widx[n] * 2 + c + 1) * D]

            # Packed biases: [128, 4] = (bq c0, bq c1, bk c0, bk c1).
            b4 = cpool.tile([128, 4], F32, tag="b4", name="b4")
            bqc = [b4[:, c:c + 1] for c in range(2)]
            bkc = [b4[:, 2 + c:3 + c] for c in range(2)]

            # ---- DMA issue order = transfer order (single serial HWDGE +
            # DMA-engine chain, ~3 DMAs in flight globally).
            xkvT_cs = xkvT_d.rearrange("p (c s) -> p c s", c=2)

            def dma_xT(i):
                a, b = XRANGES[i]
                nc.sync.dma_start(
                    xkvTt[i].rearrange("p (c s) -> p c s", c=2),
                    xkvT_cs[:, :, a:b])

            # Weights/biases go out on the ACT sequencer's HWDGE ring (TRN2
            # has two physical rings: qSPDynamicHW + qActDynamicHW), so their
            # issue chain runs in parallel with the x^T stream on SP.
            nc.scalar.dma_start(
                wqkv_sb.rearrange("p (n j c) -> p n j c", n=3, j=2),
                wqkv_g[:])
            nc.scalar.dma_start(b4[:], bqk_pnc)
            dma_xT(0)
            dma_xT(1)
            dma_xT(2)
            dma_xT(3)
            nc.sync.dma_start(
                wo_sb.rearrange("p (j c) -> p j c", j=2), _r(wo_g[:]))

            # bo broadcast across partitions on the (idle) GPSIMD engine:
            # row DMA [1,256] then partition-broadcast into both halves of
            # bob [128,512], so one wide add covers two output row-tiles.
            # No PE involvement — an fp32 ones-matmul here would let the
            # scheduler gate the attention stream on this late DMA. (bv is
            # folded into bo host-side: attention rows sum to 1.)
            bob = cpool.tile([128, 2 * D], F32, tag="bob", name="bob")
            row = cpool.tile([1, D], F32, tag="bor", name="bor")
            nc.sync.dma_start(row[:], bo_row[:])
            for half in range(2):
                nc.gpsimd.partition_broadcast(
                    bob[:, half * D:(half + 1) * D], row[:])

            # PE p-state warm-up: ~170 dummy DoubleRow matmuls on the ones8
            # tile keep the PE continuously busy from ~1.3us (after the ones8
            # memset) until the first x^T slab + weights land (~6us). The PE
            # clock ramps 0.65 -> 1.2 -> 2.4 GHz over 3us of CONTINUOUS
            # execution and resets on idle, so without this the whole first
            # ~3us of projections would run at half clock.
            ones8_3 = ones8[:].rearrange("p (two e) -> p two e", two=2)
            warm = psmm.tile([128, D], F32, tag="fp", name="fp", bufs=1)
            for _ in range(190):
                nc.tensor.matmul(warm[:, 0:128], ones8_3, ones8_3,
                                 start=True, stop=True, perf_mode=DR)

            # ---- phases 1+2, interleaved per x group ----
            # Each [128,1024] PSUM tile is processed as two independent
            # halves: matmuls for half h, then its eviction — half 0 always
            # on DVE, half 1 always on ACT, each into its own page tile. The
            # eviction of half 0 completes while the PE fills half 1, so the
            # 2-deep PSUM ring never stalls the PE.
            def evict1(dsl, src, h, bias=None):
                if bias is None:
                    if h == 0:
                        nc.vector.tensor_copy(dsl, src)
                    else:
                        nc.scalar.copy(dsl, src)
                elif h == 0:
                    nc.vector.tensor_scalar_add(dsl, src, bias)
                else:
                    nc.scalar.activation(
                        dsl, src, mybir.ActivationFunctionType.Identity,
                        bias=bias)

            def xT(c, col, width=512):  # read helper over x^T slabs
                for t, (a, b) in enumerate(XRANGES):
                    if col < b:
                        off = col - a
                        w = b - a
                        return xkvTt[t][:, c * w + off:c * w + off + width]

            def project_qk(wn, dstTp, bcol, g, et):
                # lhsT = W[d_chunk, e_tile], moving = x^T[d_chunk, group g].
                pp = psmm.tile([128, 1024], F32, tag="sp", name="sp",
                               bufs=2)
                for h in range(2):
                    blk = g * 2 + h
                    for c in range(2):
                        nc.tensor.matmul(
                            pp[:, h * 512:(h + 1) * 512],
                            wchunk(wn, c)[:, et * 128:(et + 1) * 128],
                            xT(c, blk * 512),
                            start=(c == 0), stop=(c == 1),
                        )
                    evict1(dstTp[et][blk][:],
                           pp[:, h * 512:(h + 1) * 512],
                           h, bias=bcol[et])

            def project_v(stp):
                # V natural layout [sk, e]; four sk-tiles per [128,1024]
                # PSUM tile; each half's eviction converts fp32 -> e4m3 into
                # one v8 page (= one k-tile pair, the PV stationary unit).
                vp = psmm.tile([128, 1024], F32, tag="sp", name="sp", bufs=2)
                for h in range(2):
                    for quad in (h * 2, h * 2 + 1):
                        st = stp * 4 + quad
                        for c in range(2):
                            nc.tensor.matmul(
                                vp[:, quad * D:(quad + 1) * D],
                                xT(c, st * 128, 128),
                                wchunk("wv", c),
                                start=(c == 0), stop=(c == 1),
                            )
                    evict1(v8p[stp * 2 + h][:],
                           vp[:, h * 512:(h + 1) * 512], h)

            # Strictly slab-major: all tiles reading x^T slab i are emitted
            # together, in DMA arrival order, so the PE consumes each slab
            # at the rate the chained DMA queue delivers them. (Interleaving
            # projections into the score stream was tried and loses: with the
            # 2-deep PSUM ring, consecutive score tiles end up 2 allocations
            # apart and an interposed projection tile provides less cover
            # than attention's own S+2xPV stream for the ~1.4us exp release.)
            if phases >= 2:
                project_qk("wk", kTp, bkc, 0, 0)
                project_qk("wk", kTp, bkc, 0, 1)
                project_qk("wq", qTp, bqc, 0, 0)
                project_qk("wq", qTp, bqc, 0, 1)
                project_v(0)
                project_v(1)
                project_qk("wk", kTp, bkc, 1, 0)
                project_qk("wk", kTp, bkc, 1, 1)
                project_qk("wq", qTp, bqc, 1, 0)
                project_qk("wq", qTp, bqc, 1, 1)
                project_v(2)
                project_v(3)
                project_qk("wk", kTp, bkc, 2, 0)
                project_qk("wk", kTp, bkc, 2, 1)
                project_v(4)
                project_v(5)
                project_qk("wk", kTp, bkc, 3, 0)
                project_qk("wk", kTp, bkc, 3, 1)
                project_v(6)
                project_v(7)

            # ---- phase 3: attention ----
            # Per 512-query block: 16 k-tile pairs. Scores for a pair fill a
            # [128,1024] PSUM tile, one wide exp -> e4m3 pt tile, then P@V
            # (2 DoubleRow matmuls) + denominator (1 DoubleRow) — emitted two
            # pairs behind the scores so the exp latency stays off the PE's
            # critical path. Final projections of the previous block are
            # interleaved at pair slots 2 and 4.
            ones8_3 = ones8[:].rearrange("p (two e) -> p two e", two=2)
            pending_finals = []

            def emit_pv(t, pt, acc, accd):
                pt3 = pt[:].rearrange("p (two q) -> p two q", two=2)
                # v8 page t holds exactly the k-tile pair 2t,2t+1: [128,2,256]
                vkte = v8p[t][:].rearrange("p (kt e) -> p kt e", e=D)
                first, last = (t == 0), (t == 15)
                for h in range(2):
                    nc.tensor.matmul(
                        acc[:, h * 512:(h + 1) * 512],
                        vkte[:, :, h * 128:(h + 1) * 128],
                        pt3, start=first, stop=last, perf_mode=DR)
                nc.tensor.matmul(accd[:], ones8_3, pt3,
                                 start=first, stop=last, perf_mode=DR)

            for qb in range(SQ // 512 if phases >= 3 else 0):
                qsl = slice(qb * 512, (qb + 1) * 512)
                acc = psacc.tile([128, 1024], F32, tag="acc", name="acc")
                accd = psacc.tile([128, 512], F32, tag="accd", name="accd")
                pts = []
                for t in range(16):
                    sp = psmm.tile([128, 1024], F32, tag="sp", name="sp",
                                   bufs=2)
                    for half in range(2):
                        st = t * 2 + half
                        pg, off = divmod(st * 128, 512)
                        psl = sp[:, half * 512:(half + 1) * 512]
                        nc.tensor.matmul(psl, kTp[0][pg][:, off:off + 128],
                                         qTp[0][qb][:],
                                         start=True, stop=False)
                        nc.tensor.matmul(psl, kTp[1][pg][:, off:off + 128],
                                         qTp[1][qb][:],
                                         start=False, stop=True)
                    pt = pt_pool.tile([128, 1024], FP8, tag="pt", name="pt",
                                      bufs=4)
                    nc.scalar.activation(pt[:], sp[:],
                                         mybir.ActivationFunctionType.Exp,
                                         scale=SCALE, bias=ebias[:])
                    pts.append(pt)
                    if t >= 2:
                        emit_pv(t - 2, pts[t - 2], acc, accd)
                    # Slot 15's final pads the PE between S15 and the next
                    # block's S0 (which must wait exp15's PSUM-ring release,
                    # ~1.4us after S15 — the three PV emissions alone leave
                    # a ~460ns hole). Query-block 0 has no finals yet, and
                    # the last block's tail waits on exp15 regardless: pad
                    # with dummy fp8 matmuls so the p-state ramp never
                    # resets there.
                    if t in (2, 15) and pending_finals:
                        pending_finals.pop(0)()
                    elif t == 15:
                        wt = psmm.tile([128, D], F32, tag="fp", name="fp",
                                       bufs=1)
                        for _ in range(16):
                            nc.tensor.matmul(wt[:, 0:128], ones8_3, ones8_3,
                                             start=True, stop=True,
                                             perf_mode=DR)
                    if t == 15 and qb == SQ // 512 - 1:
                        wt = psmm.tile([128, D], F32, tag="fp", name="fp",
                                       bufs=1)
                        for _ in range(36):
                            nc.tensor.matmul(wt[:, 0:128], ones8_3, ones8_3,
                                             start=True, stop=True,
                                             perf_mode=DR)
                emit_pv(14, pts[14], acc, accd)
                emit_pv(15, pts[15], acc, accd)

                rec = ovec_pool.tile([128, 512], F32, tag="rec", name="rec")
                o = [ovec_pool.tile([128, 512], F32R, tag=f"o{e}",
                                    name=f"o{e}") for e in range(2)]
                # halves: lets the first final matmuls start ~0.8us earlier
                for hsl in (slice(0, 256), slice(256, 512)):
                    nc.vector.reciprocal(rec[:, hsl], accd[:, hsl])
                    for e in range(2):
                        nc.vector.tensor_mul(
                            o[e][:, hsl], acc[:, e * 512:(e + 1) * 512][:, hsl],
                            rec[:, hsl])

                # Final projection: two row-tiles per [128,512] staging tile,
                # one paired 256-row output DMA. Deferred into the next
                # block's score stream; the last block flushes immediately,
                # using the now-free score-pool banks (one [128,1024] tile
                # per pair = two independent accumulation regions) so the
                # four units don't serialize on the single "fp" bank.
                def make_final(qb, o, pair, flush=False):
                    def emit():
                        fo = fout_pool.tile([128, 2 * D], F32, tag="fout",
                                            name="fout")
                        # Flush: both row-tiles of the pair land in adjacent
                        # [128,256] regions of one bank, so ONE wide add
                        # covers the pair (one fewer DVE op on the tail's
                        # serial chain).
                        fpw = (psmm.tile([128, 1024], F32, tag="sp",
                                         name="sp", bufs=2) if flush else None)
                        for half in range(2):
                            t4 = pair * 2 + half
                            tsl = slice(t4 * 128, (t4 + 1) * 128)
                            fp = (fpw[:, half * D:(half + 1) * D]
                                  if flush else
                                  psmm.tile([128, D], F32, tag="fp",
                                            name="fp", bufs=1)[:])
                            for e in range(2):
                                nc.tensor.matmul(fp, _r(o[e][:, tsl]),
                                                 _r(wchunk("wo", e)),
                                                 start=(e == 0), stop=(e == 1))
                            if not flush:
                                nc.vector.tensor_add(
                                    fo[:, half * D:(half + 1) * D],
                                    fp, bob[:, 0:D])
                        if flush:
                            nc.vector.tensor_add(fo[:], fpw[:, 0:2 * D],
                                                 bob[:])
                        # The two flush DMAs go out on DIFFERENT HWDGE rings
                        # (ACT + SP) so the kernel's last two transfers don't
                        # serialize on one ring's completion chaining.
                        eng = nc.scalar if (flush and pair == 0) else nc.sync
                        eng.dma_start(out_g[qb * 2 + pair],
                                      fo.rearrange("p (j c) -> p j c", j=2))
                    return emit

                if qb == SQ // 512 - 1:
                    for pair in range(2):
                        make_final(qb, o, pair, flush=True)()
                else:
                    for pair in range(2):
                        pending_finals.append(make_final(qb, o, pair))

    nc.compile()
    return nc



_NC = None


def _get_nc():
    global _NC
    if _NC is None:
        _NC = _build()
    return _NC


class _Runner:
    """Cached jitted SPMD executor (run_bass_kernel_spmd rebuilds its jax
    closure every call, forcing a retrace; this traces once)."""

    def __init__(self, nc):
        import jax
        from jax.sharding import Mesh, PartitionSpec
        from jax.experimental.shard_map import shard_map
        from concourse import bass2jax, mybir as mb

        bass2jax.install_neuronx_cc_hook()
        self.jax = jax
        if not any("axon" in str(getattr(d, "platform", "")).lower()
                   or str(d).startswith("NC_")
                   for d in jax.devices()):
            # jax was initialized on another platform (e.g. cpu for the
            # reference); reset so the axon NeuronCores are visible.
            import jax._src.xla_bridge as xb
            jax.config.update("jax_platforms", None)
            xb._clear_backends()
            if hasattr(xb.get_backend, "cache_clear"):
                xb.get_backend.cache_clear()
            if not any("axon" in str(getattr(d, "platform", "")).lower()
                       or str(d).startswith("NC_")
                       for d in jax.devices()):
                jax.config.update("jax_platforms", "axon")
                xb._clear_backends()
                if hasattr(xb.get_backend, "cache_clear"):
                    xb.get_backend.cache_clear()
        partition_name = (nc.partition_id_tensor.name
                          if nc.partition_id_tensor else None)
        in_names, out_names, out_avals = [], [], []
        for alloc in nc.m.functions[0].allocations:
            if not isinstance(alloc, mb.MemoryLocationSet):
                continue
            name = alloc.memorylocations[0].name
            if alloc.kind == "ExternalInput":
                if name != partition_name:
                    in_names.append(name)
            elif alloc.kind == "ExternalOutput":
                out_names.append(name)
                out_avals.append(jax.core.ShapedArray(
                    tuple(alloc.tensor_shape), mb.dt.np(alloc.dtype)))
        self.in_names, self.out_names, self.out_avals = \
            in_names, out_names, out_avals
        n_params, n_outs = len(in_names), len(out_names)
        bind_in_names = in_names + out_names + (
            [partition_name] if partition_name else [])

        def _body(*args):
            operands = list(args)
            if partition_name is not None:
                operands.append(bass2jax.partition_id_tensor())
            outs = bass2jax._bass_exec_p.bind(
                *operands,
                out_avals=tuple(out_avals),
                in_names=tuple(bind_in_names),
                out_names=tuple(out_names),
                lowering_input_output_aliases=(),
                sim_require_finite=True,
                sim_require_nnan=True,
                nc=nc,
            )
            return tuple(outs)

        devices = jax.devices()[:NCORES]
        mesh = Mesh(np.asarray(devices), ("core",))
        spec = (PartitionSpec("core"),) * (n_params + n_outs)
        self.fn = jax.jit(
            shard_map(_body, mesh=mesh, in_specs=spec,
                      out_specs=(PartitionSpec("core"),) * n_outs,
                      check_rep=False),
            donate_argnums=tuple(range(n_params, n_params + n_outs)),
            keep_unused=True,
        )

    def run(self, in_maps):
        concat_in = [
            np.concatenate([np.asarray(m[n]) for m in in_maps], axis=0)
            for n in self.in_names
        ]
        concat_zeros = [
            np.zeros((NCORES * a.shape[0], *a.shape[1:]), a.dtype)
            for a in self.out_avals
        ]
        outs = self.fn(*concat_in, *concat_zeros)
        return [
            {n: np.asarray(outs[i]).reshape(NCORES, *self.out_avals[i].shape)[c]
             for i, n in enumerate(self.out_names)}
            for c in range(NCORES)
        ]


_RUNNER = None


def _get_runner():
    global _RUNNER
    if _RUNNER is None:
        _RUNNER = _Runner(_get_nc())
    return _RUNNER


def kernel(**inputs):
    import ml_dtypes
    bf16 = ml_dtypes.bfloat16
    x = np.ascontiguousarray(np.asarray(inputs["x"], dtype=np.float32))
    Wq = np.asarray(inputs["Wq"], dtype=np.float32)
    Wk = np.asarray(inputs["Wk"], dtype=np.float32)
    Wv = np.asarray(inputs["Wv"], dtype=np.float32)
    Wo = np.ascontiguousarray(np.asarray(inputs["Wo"], dtype=np.float32))
    bq = np.asarray(inputs["bq"], dtype=np.float32)
    bk = np.asarray(inputs["bk"], dtype=np.float32)
    bv = np.ascontiguousarray(np.asarray(inputs["bv"], dtype=np.float32))
    bo = np.ascontiguousarray(np.asarray(inputs["bo"], dtype=np.float32))

    try:
        runner = _get_runner()
    except Exception:
        runner = None
    # bv folds into bo: attention rows sum to 1, so attn@(v+bv) = attn@v + bv.
    bo_eff = (bv @ Wo + bo).astype(np.float32)
    wqkv = np.ascontiguousarray(
        np.concatenate([Wq, Wk, Wv], axis=0).astype(bf16))
    bqk = np.ascontiguousarray(np.concatenate([bq, bk]).astype(np.float32))
    in_maps = []
    for c in range(NCORES):
        b, h = divmod(c, 2)
        # Rotate the batch so this core's queries are rows 0..SQ-1; keys and
        # values see all rows either way (softmax is key-order invariant).
        xb = (x[b] if h == 0 else
              np.concatenate([x[b, SQ:], x[b, :SQ]]))
        xbT = xb.T.astype(bf16)  # [256, 4096]
        # Pack [128, 2*S]: partition p = [chunk0 row p | chunk1 row p].
        xbTp = np.ascontiguousarray(
            np.concatenate([xbT[:128], xbT[128:]], axis=1))
        in_maps.append({
            "xkvT": xbTp, "wqkv": wqkv, "wo": Wo, "bqk": bqk, "bo": bo_eff,
        })
    results = None
    if runner is not None:
        try:
            results = runner.run(in_maps)
        except Exception:
            results = None
    if results is None:
        results = run_bass_kernel_spmd(
            _get_nc(), in_maps, core_ids=list(range(NCORES))).results
    outp = np.empty((B, S, D), dtype=np.float32)
    for c in range(NCORES):
        b, h = divmod(c, 2)
        outp[b, h * SQ:(h + 1) * SQ] = results[c]["out"]
    return outp
